# revision 20
# baseline (speedup 1.0000x reference)
"""Trainium2 Bass kernel for nn_AttentionStem (sparse local attention stem).

Math per output element (b, c, h, w), window kk = (di, dj) in 4x4, PAD=2:
  E[c,h,w]   = (emb_a[c,w] + emb_b[c,h]) * emb_mix[c,h,w]
  e1_kk      = exp(v_kk^2 * E)                  (softmax-1 numerator)
  q'         = q / sum_kk(e1)                   (fold softmax-1 denom into q)
  e2_kk      = exp(q' * k_kk * e1_kk)           (softmax-2 numerator)
  out        = sum_kk(e2 * v_kk) / sum_kk(e2)

Sharding: pure data parallel, one batch element per NeuronCore (8 cores).
Layout per core: SBUF partition p = 64*half + c  (half = h<64 ? 0 : 1),
free dims stream (h, w); KK tiles are [128, KK, n] with kk outermost.
The three sum_kk reductions run on the TensorEngine as chains of 16
PSUM-accumulating identity matmuls (exact fp32 sums, same layout out).
16-bit tensors use fp16 where the value range allows and bf16 where the
unnormalized exp(t1) magnitudes (~e^56) require the wider exponent; DVE
instructions are split per (di, dj-parity) so both operands of every
tensor_tensor keep 4-byte alignment, which the DVE 2x packed mode needs.
The KK pipeline t1 -> e1 -> f -> s2 -> e2 runs IN PLACE in one f16-sized
tile per chunk (bitcast views for the bf16 stage); m2 uses a second tile
so DVE's m2 isn't serialized behind PE's r2 reads.  Shifted B maps are
built by SBUF->SBUF DMA (engines otherwise idle), freeing GPSIMD.
"""
import sys, os
for _p in ("/opt/trn_rl_repo", "/root/.axon_site/_ro/trn_rl_repo"):
    if os.path.isdir(_p) and _p not in sys.path:
        sys.path.insert(0, _p)

from contextlib import ExitStack, nullcontext as _nullcm
import numpy as np

import concourse.bass as bass
import concourse.bacc as bacc
import concourse.tile as tile
from concourse import mybir
import concourse.bass_utils as bass_utils
from concourse.bass_types import AP
from concourse import masks

N_CORES = 8
B, CIN, H, W = 8, 3, 128, 128
C = 64
K, PAD, KK = 4, 2, 16
HP, WP = H + 2 * PAD, W + 2 * PAD  # 132, 132
HH = H // 2                        # rows per half (64)

F32 = mybir.dt.float32
BF16 = mybir.dt.bfloat16
F16 = mybir.dt.float16
F32R = mybir.dt.float32r
MULT = mybir.AluOpType.mult
ADD = mybir.AluOpType.add
EXP = mybir.ActivationFunctionType.Exp
SQUARE = mybir.ActivationFunctionType.Square

CH = 4  # h-rows per half per chunk

# Precision / engine configuration.
#   kk:    dtype of the KK-expanded pipeline (maps, t1/e1/m1/s2/e2/m2)
#   conv:  dtype of the 1x1-conv matmuls (fp32: 4 cyc/row, fp32r/bf16: 1)
#   e2_fp32: keep softmax-2 numerators in fp32 (accuracy of the output path)
# Per-tensor dtypes of the KK pipeline. fp16 where the value range allows
# (8x finer mantissa than bf16); bf16 where unnormalized exp(t1) magnitudes
# (up to ~e^56) must be representable (e1, m1, and the q/sum(e1) scale qp).
DTS_F16 = dict(map=F16, E=F16, t1=F16, e1=BF16, qp=BF16, m1=F16, s2=F16,
               e2=F16, m2=F16, mix=F16)
DTS_BF16 = {k: BF16 for k in DTS_F16}
DTS_F32 = {k: F32 for k in DTS_F16}
CFG = dict(kk=BF16, dts=DTS_F16, conv=F16, e2_fp32=False, mh=16, pool_tt=())


def _ap(base: AP, offset: int, dims):
    """Build a custom free-dim AP on a tile/dram AP, keeping its partition dim."""
    return AP(tensor=base.tensor, offset=base.offset + offset,
              ap=[list(base.ap[0])] + [list(d) for d in dims])


def build_kernel(nc, ch: int = CH, cfg=None, reps: int = 0):
    """reps>0 wraps the whole body in a hardware loop (for benchmarking)."""
    cfg = dict(CFG if cfg is None else cfg)
    f32 = F32
    dts = dict(cfg.get("dts") or {k: cfg["kk"] for k in DTS_F16})
    d_e2 = f32 if cfg["e2_fp32"] else dts["e2"]
    dkk = dts["t1"]
    dcv = cfg["conv"]               # conv matmul dtype
    split = dts["t1"] != f32       # parity-split DVE instrs for 2x mode
    pool_tt = set(cfg.get("pool_tt") or ())
    n = ch * W                      # spatial elems per partition per chunk
    mh = cfg.get("mh", 8)           # map super-chunk rows per half
    RWm = (mh + K - 1) * WP         # map cols per half per super-chunk
    piece = -(-RWm // -(-RWm // 512))           # matmul col piece (<=512)

    xp_d = nc.dram_tensor("xp", [CIN, HP * WP], dcv, kind="ExternalInput").ap()
    w_d = {}
    for nm in ("q", "k", "v"):
        for hb in "AB":
            w_d[nm + hb] = nc.dram_tensor(f"{nm}_w{hb}", [CIN, 128], dcv,
                                          kind="ExternalInput").ap()
    ea_d = nc.dram_tensor("emb_a", [C, W], f32, kind="ExternalInput").ap()
    eb_d = nc.dram_tensor("emb_b", [C, H], f32, kind="ExternalInput").ap()
    em_d = nc.dram_tensor("emb_mix", [C, H * W], dts["mix"], kind="ExternalInput").ap()
    out_d = nc.dram_tensor("out", [C, H * W], f32, kind="ExternalOutput").ap()

    with tile.TileContext(nc) as tc, ExitStack() as ctx:
        const = ctx.enter_context(tc.tile_pool(name="const", bufs=1))
        xp_p = ctx.enter_context(tc.tile_pool(name="xp", bufs=2))
        mix_p = ctx.enter_context(tc.tile_pool(name="mix", bufs=4))
        map_p = ctx.enter_context(tc.tile_pool(name="maps", bufs=2))
        kkA_p = ctx.enter_context(tc.tile_pool(name="kkA", bufs=3))
        kkB_p = ctx.enter_context(tc.tile_pool(name="kkB", bufs=2))
        sm_p = ctx.enter_context(tc.tile_pool(name="small", bufs=2))
        ps_kv = ctx.enter_context(tc.tile_pool(name="pskv", bufs=2, space="PSUM"))
        ps_q = ctx.enter_context(tc.tile_pool(name="psq", bufs=1, space="PSUM"))
        ps_acc = ctx.enter_context(tc.tile_pool(name="psacc", bufs=1, space="PSUM"))
        ps_acc2 = ctx.enter_context(tc.tile_pool(name="psacc2", bufs=2, space="PSUM"))

        # ---- constants ----
        w_t = {}
        for key, d in w_d.items():
            wtile = const.tile([CIN, 128], dcv, tag=f"w{key}")
            nc.sync.dma_start(wtile[:], d[:])
            w_t[key] = wtile
        ea_t = const.tile([128, W], f32, tag="ea")       # emb_a[c, w], both halves
        nc.sync.dma_start(ea_t[0:C, :], ea_d[:])
        nc.sync.dma_start(ea_t[C:128, :], ea_d[:])
        eb_t = const.tile([128, HH], f32, tag="eb")      # emb_b[c, 64*half + hl]
        nc.sync.dma_start(eb_t[0:C, :], _ap(eb_d, 0, [[1, HH]]))
        nc.sync.dma_start(eb_t[C:128, :], _ap(eb_d, HH, [[1, HH]]))
        idents = {}
        for dt_ in {dts["e1"], d_e2, dts["m2"]}:
            it = const.tile([128, 128], dt_, tag=f"ident{dt_}")
            masks.make_identity(nc, it[:])
            idents[dt_] = it

        def produce_maps(mh0):
            """Superchunk map production: xp DMA, k/v conv matmuls (PE),
            PSUM->SBUF map copies + v^2 (GPSIMD), shifted B maps (DMA)."""
            xp_t = xp_p.tile([CIN, 2 * RWm], dcv, tag="xp")
            for half in (0, 1):
                nc.sync.dma_start(
                    xp_t[:, half * RWm:(half + 1) * RWm],
                    _ap(xp_d, (HH * half + mh0) * WP, [[1, RWm]]))

            k_map = map_p.tile([128, RWm], dts["map"], tag="kmap")
            v_map = map_p.tile([128, RWm], dts["map"], tag="vmap")
            v2_map = map_p.tile([128, RWm], dts["map"], tag="v2map")
            for name, dmap in (("k", k_map), ("v", v_map)):
                for pc in range(0, RWm, piece):
                    pw = min(piece, RWm - pc)
                    pt = ps_kv.tile([128, 512], f32, tag="kv")
                    for half, hb in ((0, "A"), (1, "B")):
                        nc.tensor.matmul(
                            pt[:, 0:pw], w_t[name + hb][:],
                            xp_t[:, half * RWm + pc: half * RWm + pc + pw],
                            start=(half == 0), stop=(half == 1))
                    # GPSIMD can't read PSUM -> PSUM->SBUF copies on ACT;
                    # v^2 from the f16 SBUF map on GPSIMD (otherwise idle).
                    nc.scalar.copy(dmap[:, pc:pc + pw], pt[:, 0:pw])
                    if name == "v":
                        nc.gpsimd.tensor_tensor(
                            v2_map[:, pc:pc + pw], dmap[:, pc:pc + pw],
                            dmap[:, pc:pc + pw], MULT)
            if split:
                # Shifted-by-one copies (odd-dj 4B alignment) via DMA —
                # the DMA engines are otherwise idle here.
                k_b = map_p.tile([128, RWm + 2], dts["map"], tag="kb")
                v_b = map_p.tile([128, RWm + 2], dts["map"], tag="vb")
                v2_b = map_p.tile([128, RWm + 2], dts["map"], tag="v2b")
                for a_t, b_t in ((k_map, k_b), (v_map, v_b), (v2_map, v2_b)):
                    nc.sync.dma_start(b_t[:, 1:RWm + 1], a_t[:, 0:RWm])
            else:
                k_b = v_b = v2_b = None
            return xp_t, k_map, v_map, v2_map, k_b, v_b, v2_b

        def make_E(h0):
            """E = (emb_a + emb_b) * emb_mix for one chunk (GPSIMD)."""
            mix_t = mix_p.tile([128, n], dts["mix"], tag="mix")
            for half in (0, 1):
                nc.sync.dma_start(
                    mix_t[C * half:C * (half + 1), :],
                    _ap(em_d, (HH * half + h0) * W, [[1, n]]))
            tmp_t = sm_p.tile([128, n], f32, tag="tmpE")
            nc.gpsimd.tensor_tensor(
                _ap(tmp_t[:], 0, [[W, ch], [1, W]]),
                _ap(ea_t[:], 0, [[0, ch], [1, W]]),
                _ap(eb_t[:], h0, [[1, ch], [0, W]]), ADD)
            E_t = sm_p.tile([128, n], dts["E"], tag="E", bufs=4)
            nc.gpsimd.tensor_tensor(E_t[:], tmp_t[:], mix_t[:], MULT)
            return E_t

        def sc_chunks(mh0):
            return list(range(mh0, mh0 + mh, ch))

        loop_cm = tc.For_i(0, reps, 1) if reps else _nullcm()
        with loop_cm:
            maps = produce_maps(0)
            E_ts = {h0: make_E(h0) for h0 in sc_chunks(0)}
            for mh0 in range(0, HH, mh):
                xp_t, k_map, v_map, v2_map, k_b, v_b, v2_b = maps
                h0s = sc_chunks(mh0)
                for ci, h0 in enumerate(h0s):
                    ro = (h0 - mh0) * WP   # row offset into the map tiles
                    E_t = E_ts[h0]

                    q_ps = ps_q.tile([128, 512], f32, tag="q")
                    for half, hb in ((0, "A"), (1, "B")):
                        rhs = _ap(xp_t[:],
                                  half * RWm + (h0 - mh0 + PAD) * WP + PAD,
                                  [[WP, ch], [1, W]])
                        nc.tensor.matmul(q_ps[:, 0:n], w_t["q" + hb][:],
                                         rhs, start=(half == 0), stop=(half == 1))

                    # ---- KK-expanded stages ----
                    # ISA: max 3 free dims -> one instr per di (fp32), or per
                    # (di, dj-parity) when 16-bit (keeps every operand 4B-aligned).
                    def tt_kk(op_name, out_t, make_in0, make_in1):
                        if not split:
                            for di in range(K):
                                nc.vector.tensor_tensor(
                                    _ap(out_t[:], di * K * n,
                                        [[n, K], [W, ch], [1, W]]),
                                    make_in0(di, None), make_in1(di, None), MULT)
                        else:
                            for di in range(K):
                                eng = (nc.gpsimd if (op_name, di) in pool_tt
                                       else nc.vector)
                                for par in (0, 1):
                                    eng.tensor_tensor(
                                        _ap(out_t[:], (di * K + par) * n,
                                            [[2 * n, 2], [W, ch], [1, W]]),
                                        make_in0(di, par), make_in1(di, par), MULT)

                    def win_di(m_a, m_b):
                        def f(di, par):
                            if par is None:
                                return _ap(m_a[:], ro + di * WP,
                                           [[1, K], [WP, ch], [1, W]])
                            src = m_a if par == 0 else m_b
                            return _ap(src[:], ro + di * WP + 2 * par,
                                       [[2, 2], [WP, ch], [1, W]])
                        return f

                    def bc_c(c_t):
                        def f(di, par):
                            kdim = [0, K] if par is None else [0, 2]
                            return _ap(c_t[:], 0, [kdim, [W, ch], [1, W]])
                        return f

                    def kk_slice(k_t):
                        def f(di, par):
                            if par is None:
                                return _ap(k_t[:], di * K * n,
                                           [[n, K], [W, ch], [1, W]])
                            return _ap(k_t[:], (di * K + par) * n,
                                       [[2 * n, 2], [W, ch], [1, W]])
                        return f

                    def pe_reduce(src_t, acc_t, dt_):
                        # acc[(half,c), pos] = sum_kk src[(half,c), kk*n + pos]
                        # via 16 PSUM-accumulating identity matmuls (exact fp32).
                        for kk in range(KK):
                            nc.tensor.matmul(
                                acc_t[:, 0:n], idents[dt_][:],
                                src_t[:, kk * n:(kk + 1) * n],
                                start=(kk == 0), stop=(kk == KK - 1))

                    # One in-place A tile carries t1 -> e1 -> f -> s2 -> e2:
                    # every overwrite's dependency is already implied by the
                    # dataflow (f waits on r1 via rc1; e2 is elementwise on
                    # s2).  m2 gets its own B tile so DVE's m2 isn't
                    # serialized behind PE's r2 reads of e2.
                    t1 = kkA_p.tile([128, KK * n], dts["t1"], tag="kkA")
                    tt_kk("t1", t1, win_di(v2_map, v2_b), bc_c(E_t))
                    e1 = t1[:].bitcast(dts["e1"])
                    nc.scalar.activation(e1, t1[:], EXP)

                    r1_ps = ps_acc.tile([128, 512], f32, tag="r1")
                    pe_reduce(e1, r1_ps, dts["e1"])
                    rc1 = sm_p.tile([128, n], f32, tag="rc1")
                    nc.vector.reciprocal_approx_fast(rc1[:], r1_ps[:, 0:n])
                    qp_t = sm_p.tile([128, n], dts["qp"], tag="qp")
                    nc.vector.tensor_tensor(qp_t[:], q_ps[:, 0:n], rc1[:], MULT)

                    # f = q' * e1 is bounded by |q| -> fits fp16, and the
                    # contiguous x broadcast product needs no parity split.
                    # Then s2 = f * k_window (was m1 = k*e1; s2 = m1*q').
                    f_t = t1[:].bitcast(dts["m1"])
                    nc.vector.tensor_tensor(
                        _ap(f_t, 0, [[n, KK], [W, ch], [1, W]]),
                        _ap(e1, 0, [[n, KK], [W, ch], [1, W]]),
                        _ap(qp_t[:], 0, [[0, KK], [W, ch], [1, W]]), MULT)
                    s2 = t1[:].bitcast(dts["s2"])
                    tt_kk("s2", s2, win_di(k_map, k_b), kk_slice(f_t))
                    e2 = t1[:].bitcast(d_e2)
                    nc.scalar.activation(e2, s2, EXP)

                    r2_ps = ps_acc.tile([128, 512], f32, tag="r2", bufs=2)
                    pe_reduce(e2, r2_ps, d_e2)
                    m2 = kkB_p.tile([128, KK * n], dts["m2"], tag="kkB")
                    tt_kk("m2", m2, kk_slice(e2), win_di(v_map, v_b))
                    r3_ps = ps_acc2.tile([128, 512], f32, tag="r3")
                    pe_reduce(m2, r3_ps, dts["m2"])

                    rc2 = sm_p.tile([128, n], f32, tag="rc2")
                    nc.vector.reciprocal_approx_fast(rc2[:], r2_ps[:, 0:n])
                    out_t = sm_p.tile([128, n], f32, tag="out")
                    nc.vector.tensor_tensor(out_t[:], r3_ps[:, 0:n], rc2[:], MULT)

                    for half in (0, 1):
                        nc.sync.dma_start(
                            _ap(out_d, (HH * half + h0) * W, [[1, n]]),
                            out_t[C * half:C * (half + 1), :])

                    # Software pipeline: emit next superchunk's map + E
                    # production after the first chunk so PE/GPSIMD fill
                    # them in mid-superchunk instead of pulsing at the
                    # boundary.
                    if ci == 0:
                        nxt = mh0 + mh if mh0 + mh < HH else (0 if reps else None)
                        if nxt is not None:
                            maps = produce_maps(nxt)
                            E_nxt = {h: make_E(h) for h in sc_chunks(nxt)}
                if mh0 + mh < HH or reps:
                    E_ts = E_nxt


_compiled_nc = None


def _get_nc():
    global _compiled_nc
    if _compiled_nc is None:
        nc = bacc.Bacc("TRN2", target_bir_lowering=False, debug=False,
                       num_devices=N_CORES)
        build_kernel(nc)
        nc.compile()
        _compiled_nc = nc
    return _compiled_nc


def _shard_inputs(x, q_w, k_w, v_w, emb_a, emb_b, emb_mix):
    cv_np = mybir.dt.np(CFG["conv"])
    xp = np.pad(x.astype(np.float32), ((0, 0), (0, 0), (PAD, PAD), (PAD, PAD)))
    xp = xp.astype(cv_np)
    def padw(wT, hb):
        full = np.zeros((CIN, 128), np.float32)
        full[:, 64 * (hb == "B"):64 * (hb == "B") + C] = wT
        return np.ascontiguousarray(full.astype(cv_np))
    common = {
        "q_wA": padw(q_w.T, "A"), "q_wB": padw(q_w.T, "B"),
        "k_wA": padw(k_w.T, "A"), "k_wB": padw(k_w.T, "B"),
        "v_wA": padw(v_w.T, "A"), "v_wB": padw(v_w.T, "B"),
        "emb_a": np.ascontiguousarray(emb_a.astype(np.float32)),
        "emb_b": np.ascontiguousarray(emb_b.astype(np.float32)),
        "emb_mix": np.ascontiguousarray(emb_mix.reshape(C, H * W).astype(mybir.dt.np((CFG.get("dts") or {}).get("mix", CFG["kk"])))),
    }
    return [dict(common, xp=np.ascontiguousarray(xp[b].reshape(CIN, HP * WP)))
            for b in range(B)]


def kernel(x, q_w, k_w, v_w, emb_a, emb_b, emb_mix):
    x, q_w, k_w, v_w, emb_a, emb_b, emb_mix = (
        np.asarray(a, dtype=np.float32)
        for a in (x, q_w, k_w, v_w, emb_a, emb_b, emb_mix))
    nc = _get_nc()
    in_maps = _shard_inputs(x, q_w, k_w, v_w, emb_a, emb_b, emb_mix)
    res = bass_utils.run_bass_kernel_spmd(nc, in_maps, list(range(N_CORES)))
    out = np.stack([res.results[b]["out"].reshape(C, H, W) for b in range(B)])
    return out.astype(np.float32)



# revision 25
# speedup vs baseline: 1.0679x; 1.0679x over previous
"""Trainium2 Bass kernel for nn_AttentionStem (sparse local attention stem).

Math per output element (b, c, h, w), window kk = (di, dj) in 4x4, PAD=2:
  E[c,h,w]   = (emb_a[c,w] + emb_b[c,h]) * emb_mix[c,h,w]
  e1_kk      = exp(v_kk^2 * E)                  (softmax-1 numerator)
  q'         = q / sum_kk(e1)                   (fold softmax-1 denom into q)
  e2_kk      = exp(q' * k_kk * e1_kk)           (softmax-2 numerator)
  out        = sum_kk(e2 * v_kk) / sum_kk(e2)

Sharding: pure data parallel, one batch element per NeuronCore (8 cores).
Layout per core: SBUF partition p = 64*half + c  (half = h<64 ? 0 : 1),
free dims stream (h, w); KK tiles are [128, KK, n] with kk outermost.
The three sum_kk reductions run on the TensorEngine as chains of 16
PSUM-accumulating identity matmuls (exact fp32 sums, same layout out).
16-bit tensors use fp16 where the value range allows and bf16 where the
unnormalized exp(t1) magnitudes (~e^56) require the wider exponent; DVE
instructions are split per (di, dj-parity) so both operands of every
tensor_tensor keep 4-byte alignment, which the DVE 2x packed mode needs.
The KK pipeline t1 -> e1 -> f -> s2 -> e2 runs IN PLACE in one f16-sized
tile per chunk (bitcast views for the bf16 stage); m2 uses a second tile
so DVE's m2 isn't serialized behind PE's r2 reads.  Shifted B maps are
built by SBUF->SBUF DMA (engines otherwise idle), freeing GPSIMD.
"""
import sys, os
for _p in ("/opt/trn_rl_repo", "/root/.axon_site/_ro/trn_rl_repo"):
    if os.path.isdir(_p) and _p not in sys.path:
        sys.path.insert(0, _p)

from contextlib import ExitStack, nullcontext as _nullcm
import numpy as np

import concourse.bass as bass
import concourse.bacc as bacc
import concourse.tile as tile
from concourse import mybir
import concourse.bass_utils as bass_utils
from concourse.bass_types import AP
from concourse import masks

N_CORES = 8
B, CIN, H, W = 8, 3, 128, 128
C = 64
K, PAD, KK = 4, 2, 16
HP, WP = H + 2 * PAD, W + 2 * PAD  # 132, 132
HH = H // 2                        # rows per half (64)

F32 = mybir.dt.float32
BF16 = mybir.dt.bfloat16
F16 = mybir.dt.float16
F32R = mybir.dt.float32r
MULT = mybir.AluOpType.mult
ADD = mybir.AluOpType.add
EXP = mybir.ActivationFunctionType.Exp
SQUARE = mybir.ActivationFunctionType.Square

CH = 4  # h-rows per half per chunk

# Precision / engine configuration.
#   kk:    dtype of the KK-expanded pipeline (maps, t1/e1/m1/s2/e2/m2)
#   conv:  dtype of the 1x1-conv matmuls (fp32: 4 cyc/row, fp32r/bf16: 1)
#   e2_fp32: keep softmax-2 numerators in fp32 (accuracy of the output path)
# Per-tensor dtypes of the KK pipeline. fp16 where the value range allows
# (8x finer mantissa than bf16); bf16 where unnormalized exp(t1) magnitudes
# (up to ~e^56) must be representable (e1, m1, and the q/sum(e1) scale qp).
DTS_F16 = dict(map=F16, E=F16, t1=F16, e1=BF16, qp=BF16, m1=F16, s2=F16,
               e2=F16, m2=F16, mix=F16)
DTS_BF16 = {k: BF16 for k in DTS_F16}
DTS_F32 = {k: F32 for k in DTS_F16}
CFG = dict(kk=BF16, dts=DTS_F16, conv=F16, e2_fp32=False, mh=16, pool_tt=())


def _ap(base: AP, offset: int, dims):
    """Build a custom free-dim AP on a tile/dram AP, keeping its partition dim."""
    return AP(tensor=base.tensor, offset=base.offset + offset,
              ap=[list(base.ap[0])] + [list(d) for d in dims])


def build_kernel(nc, ch: int = CH, cfg=None, reps: int = 0):
    """reps>0 wraps the whole body in a hardware loop (for benchmarking)."""
    cfg = dict(CFG if cfg is None else cfg)
    f32 = F32
    dts = dict(cfg.get("dts") or {k: cfg["kk"] for k in DTS_F16})
    d_e2 = f32 if cfg["e2_fp32"] else dts["e2"]
    dkk = dts["t1"]
    dcv = cfg["conv"]               # conv matmul dtype
    split = dts["t1"] != f32       # parity-split DVE instrs for 2x mode
    pool_tt = set(cfg.get("pool_tt") or ())
    n = ch * W                      # spatial elems per partition per chunk
    mh = cfg.get("mh", 8)           # map super-chunk rows per half
    RWm = (mh + K - 1) * WP         # map cols per half per super-chunk
    piece = -(-RWm // -(-RWm // 512))           # matmul col piece (<=512)

    xp_d = nc.dram_tensor("xp", [CIN, HP * WP], dcv, kind="ExternalInput").ap()
    w_d = {}
    for nm in ("q", "k", "v"):
        for hb in "AB":
            w_d[nm + hb] = nc.dram_tensor(f"{nm}_w{hb}", [CIN, 128], dcv,
                                          kind="ExternalInput").ap()
    ea_d = nc.dram_tensor("emb_a", [C, W], f32, kind="ExternalInput").ap()
    eb_d = nc.dram_tensor("emb_b", [C, H], f32, kind="ExternalInput").ap()
    em_d = nc.dram_tensor("emb_mix", [C, H * W], dts["mix"], kind="ExternalInput").ap()
    out_d = nc.dram_tensor("out", [C, H * W], f32, kind="ExternalOutput").ap()

    with tile.TileContext(nc) as tc, ExitStack() as ctx:
        const = ctx.enter_context(tc.tile_pool(name="const", bufs=1))
        xp_p = ctx.enter_context(tc.tile_pool(name="xp", bufs=2))
        mix_p = ctx.enter_context(tc.tile_pool(name="mix", bufs=4))
        map_p = ctx.enter_context(tc.tile_pool(name="maps", bufs=2))
        kkA_p = ctx.enter_context(tc.tile_pool(name="kkA", bufs=3))
        kkB_p = ctx.enter_context(tc.tile_pool(name="kkB", bufs=2))
        sm_p = ctx.enter_context(tc.tile_pool(name="small", bufs=2))
        ps_kv = ctx.enter_context(tc.tile_pool(name="pskv", bufs=2, space="PSUM"))
        ps_q = ctx.enter_context(tc.tile_pool(name="psq", bufs=1, space="PSUM"))
        ps_acc = ctx.enter_context(tc.tile_pool(name="psacc", bufs=1, space="PSUM"))
        ps_acc2 = ctx.enter_context(tc.tile_pool(name="psacc2", bufs=2, space="PSUM"))

        # ---- constants ----
        w_t = {}
        for key, d in w_d.items():
            wtile = const.tile([CIN, 128], dcv, tag=f"w{key}")
            nc.sync.dma_start(wtile[:], d[:])
            w_t[key] = wtile
        ea_t = const.tile([128, W], f32, tag="ea")       # emb_a[c, w], both halves
        nc.sync.dma_start(ea_t[0:C, :], ea_d[:])
        nc.sync.dma_start(ea_t[C:128, :], ea_d[:])
        eb_t = const.tile([128, HH], f32, tag="eb")      # emb_b[c, 64*half + hl]
        nc.sync.dma_start(eb_t[0:C, :], _ap(eb_d, 0, [[1, HH]]))
        nc.sync.dma_start(eb_t[C:128, :], _ap(eb_d, HH, [[1, HH]]))
        idents = {}
        for dt_ in {dts["e1"], d_e2, dts["m2"]}:
            it = const.tile([128, 128], dt_, tag=f"ident{dt_}")
            masks.make_identity(nc, it[:])
            idents[dt_] = it

        def produce_maps(mh0):
            """Superchunk map production: xp DMA, k/v conv matmuls (PE),
            PSUM->SBUF map copies + v^2 (GPSIMD), shifted B maps (DMA)."""
            xp_t = xp_p.tile([CIN, 2 * RWm], dcv, tag="xp")
            for half in (0, 1):
                nc.sync.dma_start(
                    xp_t[:, half * RWm:(half + 1) * RWm],
                    _ap(xp_d, (HH * half + mh0) * WP, [[1, RWm]]))

            k_map = map_p.tile([128, RWm], dts["map"], tag="kmap")
            v_map = map_p.tile([128, RWm], dts["map"], tag="vmap")
            v2_map = map_p.tile([128, RWm], dts["map"], tag="v2map")
            for name, dmap in (("k", k_map), ("v", v_map)):
                for pc in range(0, RWm, piece):
                    pw = min(piece, RWm - pc)
                    pt = ps_kv.tile([128, 512], f32, tag="kv")
                    for half, hb in ((0, "A"), (1, "B")):
                        nc.tensor.matmul(
                            pt[:, 0:pw], w_t[name + hb][:],
                            xp_t[:, half * RWm + pc: half * RWm + pc + pw],
                            start=(half == 0), stop=(half == 1))
                    # GPSIMD can't read PSUM -> PSUM->SBUF copies on ACT;
                    # v^2 from the f16 SBUF map on GPSIMD (otherwise idle).
                    nc.scalar.copy(dmap[:, pc:pc + pw], pt[:, 0:pw])
                    if name == "v":
                        nc.gpsimd.tensor_tensor(
                            v2_map[:, pc:pc + pw], dmap[:, pc:pc + pw],
                            dmap[:, pc:pc + pw], MULT)
            if split:
                # Shifted-by-one copies (odd-dj 4B alignment) via DMA —
                # the DMA engines are otherwise idle here.
                k_b = map_p.tile([128, RWm + 2], dts["map"], tag="kb")
                v_b = map_p.tile([128, RWm + 2], dts["map"], tag="vb")
                v2_b = map_p.tile([128, RWm + 2], dts["map"], tag="v2b")
                for a_t, b_t in ((k_map, k_b), (v_map, v_b), (v2_map, v2_b)):
                    nc.sync.dma_start(b_t[:, 1:RWm + 1], a_t[:, 0:RWm])
            else:
                k_b = v_b = v2_b = None
            return xp_t, k_map, v_map, v2_map, k_b, v_b, v2_b

        def make_E(h0):
            """E = (emb_a + emb_b) * emb_mix for one chunk (GPSIMD)."""
            mix_t = mix_p.tile([128, n], dts["mix"], tag="mix")
            for half in (0, 1):
                nc.sync.dma_start(
                    mix_t[C * half:C * (half + 1), :],
                    _ap(em_d, (HH * half + h0) * W, [[1, n]]))
            tmp_t = sm_p.tile([128, n], f32, tag="tmpE")
            nc.gpsimd.tensor_tensor(
                _ap(tmp_t[:], 0, [[W, ch], [1, W]]),
                _ap(ea_t[:], 0, [[0, ch], [1, W]]),
                _ap(eb_t[:], h0, [[1, ch], [0, W]]), ADD)
            E_t = sm_p.tile([128, n], dts["E"], tag="E", bufs=6)
            nc.gpsimd.tensor_tensor(E_t[:], tmp_t[:], mix_t[:], MULT)
            return E_t

        def sc_chunks(mh0):
            return list(range(mh0, mh0 + mh, ch))

        # ---- KK-expanded stage helpers ----
        # ISA: max 3 free dims -> one instr per di (fp32), or per
        # (di, dj-parity) when 16-bit (keeps every operand 4B-aligned).
        def tt_kk(op_name, out_t, make_in0, make_in1):
            if not split:
                for di in range(K):
                    nc.vector.tensor_tensor(
                        _ap(out_t[:], di * K * n, [[n, K], [W, ch], [1, W]]),
                        make_in0(di, None), make_in1(di, None), MULT)
            else:
                for di in range(K):
                    eng = (nc.gpsimd if (op_name, di) in pool_tt
                           else nc.vector)
                    for par in (0, 1):
                        eng.tensor_tensor(
                            _ap(out_t[:], (di * K + par) * n,
                                [[2 * n, 2], [W, ch], [1, W]]),
                            make_in0(di, par), make_in1(di, par), MULT)

        def win_di(m_a, m_b, ro):
            def f(di, par):
                if par is None:
                    return _ap(m_a[:], ro + di * WP,
                               [[1, K], [WP, ch], [1, W]])
                src = m_a if par == 0 else m_b
                return _ap(src[:], ro + di * WP + 2 * par,
                           [[2, 2], [WP, ch], [1, W]])
            return f

        def bc_c(c_t):
            def f(di, par):
                kdim = [0, K] if par is None else [0, 2]
                return _ap(c_t[:], 0, [kdim, [W, ch], [1, W]])
            return f

        def kk_slice(k_t):
            def f(di, par):
                if par is None:
                    return _ap(k_t, di * K * n, [[n, K], [W, ch], [1, W]])
                return _ap(k_t, (di * K + par) * n,
                           [[2 * n, 2], [W, ch], [1, W]])
            return f

        def pe_reduce(src_t, acc_t, dt_):
            # acc[(half,c), pos] = sum_kk src[(half,c), kk*n + pos]
            # via 16 PSUM-accumulating identity matmuls (exact fp32).
            for kk in range(KK):
                nc.tensor.matmul(
                    acc_t[:, 0:n], idents[dt_][:],
                    src_t[:, kk * n:(kk + 1) * n],
                    start=(kk == 0), stop=(kk == KK - 1))

        # ---- three chunk stages, software-pipelined below so no engine's
        # in-order queue head-of-line blocks on a cross-engine round trip.
        # One in-place A tile carries t1 -> e1 -> f -> s2 -> e2 (bitcast
        # views); every overwrite's dependency is implied by the dataflow.
        # m2 gets its own B tile so DVE's m2 isn't serialized behind PE's
        # r2 reads of e2.
        def stage_A(st):
            # t1 = v2_win * E; e1 = exp(t1) (ACT); r1 = sum_kk e1 (PE)
            t1 = kkA_p.tile([128, KK * n], dts["t1"], tag="kkA")
            tt_kk("t1", t1, win_di(st["v2_map"], st["v2_b"], st["ro"]),
                  bc_c(st["E_t"]))
            e1 = t1[:].bitcast(dts["e1"])
            nc.scalar.activation(e1, t1[:], EXP)
            r1_ps = ps_acc.tile([128, 512], f32, tag="r1", bufs=2)
            pe_reduce(e1, r1_ps, dts["e1"])
            st["t1"], st["e1"], st["r1"] = t1, e1, r1_ps

        def stage_B(st):
            # q conv; q' = q/sum(e1); f = e1*q'; s2 = f*k_win; e2 = exp(s2)
            # (ACT); r2 = sum_kk e2 (PE)
            q_ps = ps_q.tile([128, 512], f32, tag="q")
            for half, hb in ((0, "A"), (1, "B")):
                rhs = _ap(st["xp_t"][:],
                          half * RWm + (st["h0"] - st["mh0"] + PAD) * WP + PAD,
                          [[WP, ch], [1, W]])
                nc.tensor.matmul(q_ps[:, 0:n], w_t["q" + hb][:],
                                 rhs, start=(half == 0), stop=(half == 1))
            rc1 = sm_p.tile([128, n], f32, tag="rc1")
            nc.vector.reciprocal_approx_fast(rc1[:], st["r1"][:, 0:n])
            qp_t = sm_p.tile([128, n], dts["qp"], tag="qp")
            nc.vector.tensor_tensor(qp_t[:], q_ps[:, 0:n], rc1[:], MULT)

            t1, e1 = st["t1"], st["e1"]
            # f = q' * e1 is bounded by |q| -> fits fp16; contiguous *
            # broadcast needs no parity split.  s2 = f * k_window.
            f_t = t1[:].bitcast(dts["m1"])
            nc.vector.tensor_tensor(
                _ap(f_t, 0, [[n, KK], [W, ch], [1, W]]),
                _ap(e1, 0, [[n, KK], [W, ch], [1, W]]),
                _ap(qp_t[:], 0, [[0, KK], [W, ch], [1, W]]), MULT)
            s2 = t1[:].bitcast(dts["s2"])
            tt_kk("s2", s2, win_di(st["k_map"], st["k_b"], st["ro"]),
                  kk_slice(f_t))
            e2 = t1[:].bitcast(d_e2)
            nc.scalar.activation(e2, s2, EXP)
            r2_ps = ps_acc.tile([128, 512], f32, tag="r2", bufs=2)
            pe_reduce(e2, r2_ps, d_e2)
            st["e2"], st["r2"] = e2, r2_ps

        def stage_C(st):
            # m2 = e2*v_win; r3 = sum_kk m2 (PE); out = r3 / r2
            m2 = kkB_p.tile([128, KK * n], dts["m2"], tag="kkB")
            tt_kk("m2", m2, kk_slice(st["e2"]),
                  win_di(st["v_map"], st["v_b"], st["ro"]))
            r3_ps = ps_acc2.tile([128, 512], f32, tag="r3", bufs=1)
            pe_reduce(m2, r3_ps, dts["m2"])
            rc2 = sm_p.tile([128, n], f32, tag="rc2")
            nc.vector.reciprocal_approx_fast(rc2[:], st["r2"][:, 0:n])
            out_t = sm_p.tile([128, n], f32, tag="out")
            nc.vector.tensor_tensor(out_t[:], r3_ps[:, 0:n], rc2[:], MULT)
            for half in (0, 1):
                nc.sync.dma_start(
                    _ap(out_d, (HH * half + st["h0"]) * W, [[1, n]]),
                    out_t[C * half:C * (half + 1), :])

        loop_cm = tc.For_i(0, reps, 1) if reps else _nullcm()
        with loop_cm:
            maps_by_sc = {0: produce_maps(0)}
            E_ts = {h0: make_E(h0) for h0 in sc_chunks(0)}
            all_chunks = [(mh0, h0) for mh0 in range(0, HH, mh)
                          for h0 in sc_chunks(mh0)]
            nch = len(all_chunks)
            states = {}
            for idx in range(nch + 2):
                if idx < nch:
                    mh0, h0 = all_chunks[idx]
                    xp_t, k_map, v_map, v2_map, k_b, v_b, v2_b = \
                        maps_by_sc[mh0]
                    states[idx] = dict(
                        mh0=mh0, h0=h0, ro=(h0 - mh0) * WP, E_t=E_ts[h0],
                        xp_t=xp_t, k_map=k_map, v_map=v_map, v2_map=v2_map,
                        k_b=k_b, v_b=v_b, v2_b=v2_b)
                    stage_A(states[idx])
                if 1 <= idx and idx - 1 < nch:
                    stage_B(states[idx - 1])
                if 2 <= idx:
                    stC = states.pop(idx - 2)
                    stage_C(stC)
                    # Emit the next superchunk's map + E production right
                    # after stage C of a superchunk's first chunk: late
                    # enough that no still-unread tile shares its pool
                    # slots, two iterations before its maps are consumed.
                    if stC["h0"] == stC["mh0"]:
                        nxt = (stC["mh0"] + mh if stC["mh0"] + mh < HH
                               else (0 if reps else None))
                        if nxt is not None:
                            maps_by_sc[nxt] = produce_maps(nxt)
                            for h in sc_chunks(nxt):
                                E_ts[h] = make_E(h)


_compiled_nc = None


def _get_nc():
    global _compiled_nc
    if _compiled_nc is None:
        nc = bacc.Bacc("TRN2", target_bir_lowering=False, debug=False,
                       num_devices=N_CORES)
        build_kernel(nc)
        nc.compile()
        _compiled_nc = nc
    return _compiled_nc


def _shard_inputs(x, q_w, k_w, v_w, emb_a, emb_b, emb_mix):
    cv_np = mybir.dt.np(CFG["conv"])
    xp = np.pad(x.astype(np.float32), ((0, 0), (0, 0), (PAD, PAD), (PAD, PAD)))
    xp = xp.astype(cv_np)
    def padw(wT, hb):
        full = np.zeros((CIN, 128), np.float32)
        full[:, 64 * (hb == "B"):64 * (hb == "B") + C] = wT
        return np.ascontiguousarray(full.astype(cv_np))
    common = {
        "q_wA": padw(q_w.T, "A"), "q_wB": padw(q_w.T, "B"),
        "k_wA": padw(k_w.T, "A"), "k_wB": padw(k_w.T, "B"),
        "v_wA": padw(v_w.T, "A"), "v_wB": padw(v_w.T, "B"),
        "emb_a": np.ascontiguousarray(emb_a.astype(np.float32)),
        "emb_b": np.ascontiguousarray(emb_b.astype(np.float32)),
        "emb_mix": np.ascontiguousarray(emb_mix.reshape(C, H * W).astype(mybir.dt.np((CFG.get("dts") or {}).get("mix", CFG["kk"])))),
    }
    return [dict(common, xp=np.ascontiguousarray(xp[b].reshape(CIN, HP * WP)))
            for b in range(B)]


def kernel(x, q_w, k_w, v_w, emb_a, emb_b, emb_mix):
    x, q_w, k_w, v_w, emb_a, emb_b, emb_mix = (
        np.asarray(a, dtype=np.float32)
        for a in (x, q_w, k_w, v_w, emb_a, emb_b, emb_mix))
    nc = _get_nc()
    in_maps = _shard_inputs(x, q_w, k_w, v_w, emb_a, emb_b, emb_mix)
    res = bass_utils.run_bass_kernel_spmd(nc, in_maps, list(range(N_CORES)))
    out = np.stack([res.results[b]["out"].reshape(C, H, W) for b in range(B)])
    return out.astype(np.float32)

